# revision 36
# baseline (speedup 1.0000x reference)
"""Trainium2 kernel for nn_Attention_64235530879045.

Mathematical structure of the reference module:
  v[b,h,m,d] = spe_agg[b, h*D+d]  (broadcast over sequence m), and
  softmax rows sum to 1, so  attn @ v == v  exactly:
    out[b,h,n,d] = sum_m attn[b,h,n,m] * v[b,h,d] = v[b,h,d].
  Therefore the module output is
    y[b,n,:] = spe_agg[b] @ W_proj.T + b_proj      (independent of n, x, W_qkv)
  broadcast over the N=1024 sequence positions.

Device strategy (8 NeuronCores, no collectives):
  Tensor-parallel over output channels: core i owns columns [96*i, 96*(i+1)).
  Raw bacc (no TileContext), per core:
    1. y1 = spe_agg @ W_proj[cols].T  (8 x 96 in fp32 PSUM; K=768 in 6
       chunks of 128, bf16 operands).  Inputs arrive in 3 DMAs: SP ring
       spe+w0..2, ACT ring w3..5 then mask+bias; each DMA has its OWN
       semaphore (a later DMA's receipts can overtake an earlier one's on
       the same ring), and per-partition runs stay >= 512 B (below that
       SDMA read-modify-write costs ~4x input bandwidth).
    2. block-diagonal staging without cross-partition moves (engine APs
       must start at partition 0/32/64/96, so per-row scatter is illegal):
       a 0/1 mask is DMA'd into rhs_bd[0:8] and DVE computes, in place,
       rhs_bd[b, b', j] = y1[b, j] * (b == b'); row 8 holds the bias.
       This replaces the baseline's SBUF->SBUF DMA staging (+1.9 us
       receipt latency on the critical path).
    3. partition-broadcast: ones9[9,128].T @ rhs_bd[:, pair cols] -> one
       192-col matmul per batch pair into its own PSUM bank;
       bc[p, (b,j)] = y1[b,j] + b_proj[j] for all 128 partitions.  Pair 0
       is split per batch into two separate banks so DVE (batch 0) and
       ACT (batch 1) fan out concurrently -- concurrent engine reads of
       the SAME PSUM bank hang the device.
    4. fan-out (broadcast-cast PSUM f32 -> bf16 x4 repeats into
       osb[p, b, r, j], 768-B contiguous runs): DVE does batch 0 and
       pairs 1, 3; ACT does batch 1 and pair 2 from disjoint banks,
       running concurrently with DVE and pacing the mid-drain.
    5. 8 per-batch output DMAs (192 KB bf16 each) alternating between the
       two HWDGE rings (SP: even batches, ACT: odd): each SDMA engine
       serves each queue at only ~11.5 B/ns, so the full ~368 GB/s HBM
       drain needs both rings busy the whole time.  Each DMA dispatches
       as soon as its pair is materialized (semaphore-gated: engine
       program order does not order a dma_start's reads after a prior
       compute op's writes); source repeats each 768-B run twice
       (stride-0).  DRAM rows n = p*8 + r.
    6. Exit: the Block all-engine barrier (a ~3.5 us semaphore butterfly)
       is suppressed; every semaphore's final value happens-before
       s_out>=128, so SP's wait alone orders the trailing sem_clear.
  Host-side: reshape + concat channels; bf16 -> f32 upcast is lossless.
"""

from unittest import mock

import numpy as np
import ml_dtypes

import concourse.bass as bass
import concourse.mybir as mybir
from concourse import bacc
from concourse.bass_utils import run_bass_kernel_spmd

# bass_utils' axon trace path imports antenv.axon_hooks unconditionally when
# BASS_TRACE is set; this container's antenv stub lacks it. Provide the hook
# (real NTFF profiling when the boot module is available, else a graceful
# no-op) so tracing never crashes the kernel.
try:
    import antenv.axon_hooks  # noqa: F401
except ImportError:
    import sys as _sys
    import types as _types

    def _make_ntff_hook():
        try:
            from trn_agent_boot.trn_boot import _ntff_profile_via_ctypes
            return _ntff_profile_via_ctypes("/opt/axon/libaxon_pjrt.so")
        except Exception:
            return None

    _hook = _make_ntff_hook()
    _m = _types.ModuleType("antenv.axon_hooks")
    _m.get_axon_ntff_profile_hook = lambda: _hook
    _sys.modules["antenv.axon_hooks"] = _m

B, N, C = 8, 1024, 768
N_CORES = 8
CS = C // N_CORES          # 96 output channels per core
KC = C // 128              # 6 contraction chunks
NB = N // 128              # 8 row repeats per partition; row n = p*8 + rep
R = 4                      # physical repeats materialized in SBUF
KA = 3                                  # w chunks in the first (SP) DMA
WA_COLS = KC * B + KA * CS              # spe chunks + w chunks 0..2 (672 B
W2_COLS = (KC - KA) * CS                # rows); w chunks 3..5 (576 B rows).
# Keep per-partition DMA runs >= 512 B: below that the SDMA engines fall
# into read-modify-write and input bandwidth drops ~4x.

F32 = mybir.dt.float32
BF16 = mybir.dt.bfloat16
IN_DT = BF16
IN_NP = ml_dtypes.bfloat16

_CACHE = {}


def _build():
    # Bass.__init__ unconditionally emits 4 const-pool memsets plus an
    # all-engine barrier at the end of the preamble. This kernel uses no
    # const APs, and the startup semaphore clears are already ordered ahead
    # of user code by the NRT pseudo-barrier, so both are dead weight.
    with (
        mock.patch.object(bass.Bass, "all_engine_barrier",
                          lambda self, **kw: None),
        mock.patch.object(bass.BassGpSimd, "memset",
                          lambda self, ap, c: None, create=True),
    ):
        nc = bacc.Bacc("TRN2", target_bir_lowering=False, debug=False,
                       num_devices=N_CORES)

    wa_d = nc.dram_tensor("wa", [128, WA_COLS], IN_DT, kind="ExternalInput")
    wb_d = nc.dram_tensor("wb", [128, W2_COLS], IN_DT, kind="ExternalInput")
    mb_d = nc.dram_tensor("mb", [9, C], IN_DT, kind="ExternalInput")
    out_d = nc.dram_tensor("out", [B, 128, NB, CS], BF16,
                           kind="ExternalOutput")

    with (
        nc.sbuf_tensor([128, WA_COLS], IN_DT) as wa_sb,
        nc.sbuf_tensor([128, W2_COLS], IN_DT) as wb_sb,
        nc.sbuf_tensor([9, B, CS], IN_DT) as rhs_bd,  # rows 0-7 diag, 8 bias
        nc.sbuf_tensor([9, 128], IN_DT) as ones9,
        nc.sbuf_tensor([128, B, R, CS], BF16) as osb,
        nc.psum_tensor([128, CS], F32) as y1_ps,
        nc.psum_tensor([128, 4, 2, 256], F32) as bc_ps,  # one bank per pair
        nc.psum_tensor([128, 512], F32) as bc1_ps,    # batch 1's own bank
        nc.semaphore("s_a1") as s_a1,      # spe + w0..w2 (SP ring)
        nc.semaphore("s_b1") as s_b1,      # w3..w5 (ACT ring, 1st)
        nc.semaphore("s_mb") as s_mb,      # mask rows + bias (ACT ring, 2nd)
        nc.semaphore("s_pe") as s_pe,      # y1 done
        nc.semaphore("s_bd") as s_bd,      # block-diag copies done (per pair)
        nc.semaphore("s_bc") as s_bc,      # bc matmuls done (per pair)
        nc.semaphore("s_fod") as s_fod,    # fan-out done (DVE)
        nc.semaphore("s_foa") as s_foa,    # batch-1 fan-out done (ACT)
        nc.semaphore("s_out") as s_out,    # output DMAs done (8*16)
    ):
        # Input loads from the main BB so each engine dispatches them right
        # after its startup, ahead of the branch into its Block body.
        # One semaphore per DMA: a multi-DMA ring can deliver a later DMA's
        # sem-incs before an earlier DMA's data (a 1-partition transfer posts
        # dummy incs from engines with no data), so threshold counts on a
        # shared semaphore are racy.
        nc.sync.dma_start(out=wa_sb[:], in_=wa_d[:]).then_inc(s_a1, 16)
        nc.scalar.dma_start(out=wb_sb[:], in_=wb_d[:]).then_inc(s_b1, 16)
        # Mask rows + bias in one transfer, straight into rhs_bd; the mask
        # rows are multiplied by y1 in place later.
        nc.scalar.dma_start(
            out=rhs_bd[:].rearrange("p b j -> p (b j)"),
            in_=mb_d[:]).then_inc(s_mb, 16)

        block_cm = nc.Block(no_gpsimd_drain=True)
        block = block_cm.__enter__()

        W0 = KC * B  # w chunks start here in wa_sb

        def out_src(b):
            return (osb[:, b]
                    .rearrange("p c j -> p (c j)")
                    .unsqueeze(1).broadcast_to([128, NB // R, R * CS]))

        @block.tensor
        def _(pe):
            pe.wait_ge(s_a1, 16)
            for k in range(KA):
                nc.tensor.matmul(
                    y1_ps[:B, :], wa_sb[:, k * B:(k + 1) * B],
                    wa_sb[:, W0 + k * CS:W0 + (k + 1) * CS],
                    start=(k == 0), stop=False,
                )
            pe.wait_ge(s_b1, 16)
            for k in range(KA, KC):
                mm = nc.tensor.matmul(
                    y1_ps[:B, :], wa_sb[:, k * B:(k + 1) * B],
                    wb_sb[:, (k - KA) * CS:(k - KA + 1) * CS],
                    start=False, stop=(k == KC - 1),
                )
            mm.then_inc(s_pe, 1)
            pe.wait_ge(s_mb, 16)   # bias row landed
            # Pair 0 is split per batch into two PSUM banks so DVE (batch 0)
            # and ACT (batch 1) can fan out concurrently without touching the
            # same bank.
            pe.wait_ge(s_bd, 1)
            nc.tensor.matmul(
                bc_ps[:, 0, 0:1, 0:CS], ones9[:], rhs_bd[:, 0:1, :],
                start=True, stop=True,
            ).then_inc(s_bc, 1)
            nc.tensor.matmul(
                bc1_ps[:, 0:CS], ones9[:],
                rhs_bd[:, 1:2, :].rearrange("k a j -> k (a j)"),
                start=True, stop=True,
            ).then_inc(s_bc, 1)
            for p in range(1, 4):
                pe.wait_ge(s_bd, 2)  # remaining diag cols ready
                nc.tensor.matmul(
                    bc_ps[:, p, :, 0:CS], ones9[:],
                    rhs_bd[:, 2 * p:2 * p + 2, :],
                    start=True, stop=True,
                ).then_inc(s_bc, 1)

        @block.vector
        def _(dve):
            # ones9 first: its completion is ordered before PE's LDWEIGHTS
            # through the in-order DVE pipeline and the s_bd chain.
            nc.vector.memset(ones9[:], 1.0)
            dve.wait_ge(s_mb, 16)  # mask rows landed (in rhs_bd)
            dve.wait_ge(s_pe, 1)
            # Engine APs cannot start at arbitrary partitions, so the block
            # diagonal cannot be written row-by-row.  Instead build it in two
            # legal ops: rhs_bd[b, b', j] = y1[b, j] * mask[b, b', j] with
            # mask[b, b', j] = (b == b'), all partition-base-0.
            # Uneven split: pair 0's columns first so bc0 / fan0 / the first
            # output dispatch start ~0.4 us earlier; the rest in one op.
            for lo, hi in ((0, 2), (2, 8)):
                nc.vector.tensor_mul(
                    rhs_bd[0:8, lo:hi, :],
                    y1_ps[:B, :].unsqueeze(1).broadcast_to([8, hi - lo, CS]),
                    rhs_bd[0:8, lo:hi, :]).then_inc(s_bd, 1)
            dve.wait_ge(s_bc, 1)
            fan = (bc_ps[:, 0, 0:1, 0:CS]
                   .unsqueeze(2).broadcast_to([128, 1, R, CS]))
            nc.vector.tensor_copy(osb[:, 0:1], fan).then_inc(s_fod, 1)
            for p in (1, 3):   # pair 2's fan runs on ACT (its own bank)
                dve.wait_ge(s_bc, p + 2)
                fan = (bc_ps[:, p, :, 0:CS]
                       .unsqueeze(2).broadcast_to([128, 2, R, CS]))
                nc.vector.tensor_copy(osb[:, 2 * p:2 * p + 2], fan).then_inc(
                    s_fod, 1)

        # Output batches alternate between the two HWDGE rings (SP: even,
        # ACT: odd): a single ring only sustains ~184 GB/s (each SDMA engine
        # serves each queue at ~11.5 B/ns), so the full ~368 GB/s needs both
        # rings busy for the whole drain.
        @block.scalar
        def _(act):
            act.wait_ge(s_bc, 2)
            fan = (bc1_ps[:, 0:CS].unsqueeze(1).unsqueeze(2)
                   .broadcast_to([128, 1, R, CS]))
            nc.scalar.copy(osb[:, 1:2], fan).then_inc(s_foa, 1)
            # wait on the copy's semaphore: program order does not order a
            # dma_start's reads after a prior compute op's writes.
            act.wait_ge(s_foa, 1)
            act.dma_start(out=out_d[1], in_=out_src(1)).then_inc(s_out, 16)
            act.wait_ge(s_bc, 4)
            fan = (bc_ps[:, 2, :, 0:CS]
                   .unsqueeze(2).broadcast_to([128, 2, R, CS]))
            nc.scalar.copy(osb[:, 4:6], fan).then_inc(s_foa, 1)
            act.wait_ge(s_fod, 2)   # DVE fan1 done -> batches 2,3
            act.dma_start(out=out_d[3], in_=out_src(3)).then_inc(s_out, 16)
            act.wait_ge(s_foa, 2)   # own fan2 done -> batches 4,5
            act.dma_start(out=out_d[5], in_=out_src(5)).then_inc(s_out, 16)
            act.wait_ge(s_fod, 3)   # DVE fan3 done -> batches 6,7
            act.dma_start(out=out_d[7], in_=out_src(7)).then_inc(s_out, 16)

        @block.sync
        def _(sp):
            sp.wait_ge(s_fod, 1)
            sp.dma_start(out=out_d[0], in_=out_src(0)).then_inc(s_out, 16)
            sp.wait_ge(s_fod, 2)
            sp.dma_start(out=out_d[2], in_=out_src(2)).then_inc(s_out, 16)
            sp.wait_ge(s_foa, 2)
            sp.dma_start(out=out_d[4], in_=out_src(4)).then_inc(s_out, 16)
            sp.wait_ge(s_fod, 3)
            sp.dma_start(out=out_d[6], in_=out_src(6)).then_inc(s_out, 16)
            sp.wait_ge(s_out, 128)

        # Block exit emits per-engine drains + an all-engine barrier.  The
        # barrier's butterfly costs ~3.5 us of semaphore ping-pong and is
        # redundant here: every semaphore's final value happens-before
        # s_out>=128 through the dependency chain, so SP's wait alone orders
        # the trailing sem_clear.  Suppress the barrier, keep the drains.
        with mock.patch.object(bass.Bass, "all_engine_barrier",
                               lambda self, **kw: None):
            block_cm.__exit__(None, None, None)
        sems = [s_a1, s_b1, s_mb, s_pe, s_bd, s_bc, s_fod, s_foa, s_out]
        nums = sorted(s.num for s in sems)
        assert nums == list(range(nums[0], nums[0] + len(nums)))
        nc.sync.sem_clear(range(nums[0], nums[-1] + 1))

    nc.compile()
    return nc


def _prep_inputs(spe_agg, W_proj, b_proj):
    # spe_host[p, k*B+b] = spe_agg[b, k*128+p]
    spe_host = (np.ascontiguousarray(spe_agg.T).reshape(KC, 128, B)
                .transpose(1, 0, 2).astype(IN_NP).reshape(128, KC * B))

    # mask[b, b'*CS + j] = (b == b')
    mask = np.kron(np.eye(B, dtype=np.float32),
                   np.ones((1, CS), dtype=np.float32)).astype(IN_NP)
    mask = np.ascontiguousarray(mask)

    wpt_full = np.ascontiguousarray(W_proj.T)          # (C, C): [c, j]
    in_maps = []
    for i in range(N_CORES):
        j0 = i * CS
        w = (wpt_full[:, j0:j0 + CS].reshape(KC, 128, CS)
             .transpose(1, 0, 2))                       # (128, KC, CS)
        wa = np.concatenate(
            [spe_host, w[:, :KA].reshape(128, KA * CS).astype(IN_NP)], axis=1)
        wb = np.ascontiguousarray(
            w[:, KA:].reshape(128, W2_COLS)).astype(IN_NP)
        mb = np.concatenate(
            [mask, np.tile(b_proj[j0:j0 + CS], B)[None, :].astype(IN_NP)],
            axis=0)
        in_maps.append({"wa": np.ascontiguousarray(wa),
                        "wb": wb, "mb": np.ascontiguousarray(mb)})
    return in_maps


def kernel(x, spe_agg, W_qkv, W_proj, b_proj):
    # x and W_qkv do not affect the output (see module analysis above).
    spe_agg = np.ascontiguousarray(spe_agg, dtype=np.float32)
    W_proj = np.ascontiguousarray(W_proj, dtype=np.float32)
    b_proj = np.ascontiguousarray(b_proj, dtype=np.float32)

    if "nc" not in _CACHE:
        _CACHE["nc"] = _build()
    nc = _CACHE["nc"]

    in_maps = _prep_inputs(spe_agg, W_proj, b_proj)
    res = run_bass_kernel_spmd(nc, in_maps, core_ids=list(range(N_CORES)))
    # per-core out: (B, 128, NB, CS) with row n = p*8 + r -> (B, N, CS).
    # Device writes bf16; the values are exactly bf16-representable, so the
    # host f32 upcast is lossless.
    shards = [np.asarray(res.results[i]["out"]).astype(np.float32)
              .reshape(B, N, CS) for i in range(N_CORES)]
    return np.concatenate(shards, axis=2)


# revision 37
# speedup vs baseline: 1.1073x; 1.1073x over previous
"""Trainium2 kernel for nn_Attention_64235530879045.

Mathematical structure of the reference module:
  v[b,h,m,d] = spe_agg[b, h*D+d]  (broadcast over sequence m), and
  softmax rows sum to 1, so  attn @ v == v  exactly:
    out[b,h,n,d] = sum_m attn[b,h,n,m] * v[b,h,d] = v[b,h,d].
  Therefore the module output is
    y[b,n,:] = spe_agg[b] @ W_proj.T + b_proj      (independent of n, x, W_qkv)
  broadcast over the N=1024 sequence positions.

Device strategy (8 NeuronCores, no collectives):
  Tensor-parallel over output channels: core i owns columns [96*i, 96*(i+1)).
  Raw bacc (no TileContext), per core:
    1. y1 = spe_agg @ W_proj[cols].T  (8 x 96 in fp32 PSUM; K=768 in 6
       chunks of 128, bf16 operands).  Inputs arrive in 3 DMAs: SP ring
       spe+w0..2, ACT ring w3..5 then mask+bias; each DMA has its OWN
       semaphore (a later DMA's receipts can overtake an earlier one's on
       the same ring), and per-partition runs stay >= 512 B (below that
       SDMA read-modify-write costs ~4x input bandwidth).
    2. block-diagonal staging without cross-partition moves (engine APs
       must start at partition 0/32/64/96, so per-row scatter is illegal):
       a 0/1 mask is DMA'd into rhs_bd[0:8] and DVE computes, in place,
       rhs_bd[b, b', j] = y1[b, j] * (b == b'); row 8 holds the bias.
       This replaces the baseline's SBUF->SBUF DMA staging (+1.9 us
       receipt latency on the critical path).
    3. partition-broadcast: ones9[9,128].T @ rhs_bd[:, pair cols] -> one
       192-col matmul per batch pair into its own PSUM bank;
       bc[p, (b,j)] = y1[b,j] + b_proj[j] for all 128 partitions.  Pair 0
       is split per batch into two separate banks so DVE (batch 0) and
       ACT (batch 1) fan out concurrently -- concurrent engine reads of
       the SAME PSUM bank hang the device.
    4. fan-out (broadcast-cast PSUM f32 -> bf16 x4 repeats into
       osb[p, b, r, j], 768-B contiguous runs): DVE does batch 0 and
       pairs 1, 3; ACT does batch 1 and pair 2 from disjoint banks,
       running concurrently with DVE and pacing the mid-drain.
    5. 8 per-batch output DMAs (192 KB bf16 each) alternating between the
       two HWDGE rings (SP: even batches, ACT: odd): each SDMA engine
       serves each queue at only ~11.5 B/ns, so the full ~368 GB/s HBM
       drain needs both rings busy the whole time.  Each DMA dispatches
       as soon as its pair is materialized (semaphore-gated: engine
       program order does not order a dma_start's reads after a prior
       compute op's writes); source repeats each 768-B run twice
       (stride-0).  DRAM rows n = p*8 + r.
    6. Exit: the Block all-engine barrier (a ~3.5 us semaphore butterfly)
       is suppressed; every semaphore's final value happens-before
       s_out>=128, so SP's wait alone orders the trailing sem_clear.
  Host-side: reshape + concat channels; bf16 -> f32 upcast is lossless.
"""

from unittest import mock

import numpy as np
import ml_dtypes

import concourse.bass as bass
import concourse.mybir as mybir
from concourse import bacc
from concourse.bass_utils import run_bass_kernel_spmd

# bass_utils' axon trace path imports antenv.axon_hooks unconditionally when
# BASS_TRACE is set; this container's antenv stub lacks it. Provide the hook
# (real NTFF profiling when the boot module is available, else a graceful
# no-op) so tracing never crashes the kernel.
try:
    import antenv.axon_hooks  # noqa: F401
except ImportError:
    import sys as _sys
    import types as _types

    def _make_ntff_hook():
        try:
            from trn_agent_boot.trn_boot import _ntff_profile_via_ctypes
            return _ntff_profile_via_ctypes("/opt/axon/libaxon_pjrt.so")
        except Exception:
            return None

    _hook = _make_ntff_hook()
    _m = _types.ModuleType("antenv.axon_hooks")
    _m.get_axon_ntff_profile_hook = lambda: _hook
    _sys.modules["antenv.axon_hooks"] = _m

B, N, C = 8, 1024, 768
N_CORES = 8
CS = C // N_CORES          # 96 output channels per core
KC = C // 128              # 6 contraction chunks
NB = N // 128              # 8 row repeats per partition; row n = p*8 + rep
R = 4                      # physical repeats materialized in SBUF
KA = 3                                  # w chunks in the first (SP) DMA
WA_COLS = KC * B + KA * CS              # spe chunks + w chunks 0..2 (672 B
W2_COLS = (KC - KA) * CS                # rows); w chunks 3..5 (576 B rows).
# Keep per-partition DMA runs >= 512 B: below that the SDMA engines fall
# into read-modify-write and input bandwidth drops ~4x.

F32 = mybir.dt.float32
BF16 = mybir.dt.bfloat16
IN_DT = BF16
IN_NP = ml_dtypes.bfloat16

_CACHE = {}


def _build():
    # Bass.__init__ unconditionally emits 4 const-pool memsets plus an
    # all-engine barrier at the end of the preamble. This kernel uses no
    # const APs, and the startup semaphore clears are already ordered ahead
    # of user code by the NRT pseudo-barrier, so both are dead weight.
    with (
        mock.patch.object(bass.Bass, "all_engine_barrier",
                          lambda self, **kw: None),
        mock.patch.object(bass.BassGpSimd, "memset",
                          lambda self, ap, c: None, create=True),
    ):
        nc = bacc.Bacc("TRN2", target_bir_lowering=False, debug=False,
                       num_devices=N_CORES)

    wa_d = nc.dram_tensor("wa", [128, WA_COLS], IN_DT, kind="ExternalInput")
    wb_d = nc.dram_tensor("wb", [128, W2_COLS], IN_DT, kind="ExternalInput")
    mb_d = nc.dram_tensor("mb", [9, C], IN_DT, kind="ExternalInput")
    out_d = nc.dram_tensor("out", [B, 128, NB, CS], BF16,
                           kind="ExternalOutput")

    with (
        nc.sbuf_tensor([128, WA_COLS], IN_DT) as wa_sb,
        nc.sbuf_tensor([128, W2_COLS], IN_DT) as wb_sb,
        nc.sbuf_tensor([9, B, CS], IN_DT) as rhs_bd,  # rows 0-7 diag, 8 bias
        nc.sbuf_tensor([9, 128], IN_DT) as ones9,
        nc.sbuf_tensor([128, B, R, CS], BF16) as osb,
        nc.psum_tensor([128, CS], F32) as y1_ps,
        nc.psum_tensor([128, 4, 2, 256], F32) as bc_ps,  # one bank per pair
        nc.psum_tensor([128, 512], F32) as bc1_ps,    # batch 1's own bank
        nc.semaphore("s_a1") as s_a1,      # spe + w0..w2 (SP ring)
        nc.semaphore("s_b1") as s_b1,      # w3..w5 (ACT ring, 1st)
        nc.semaphore("s_mb") as s_mb,      # mask rows + bias (ACT ring, 2nd)
        nc.semaphore("s_pe") as s_pe,      # y1 done
        nc.semaphore("s_bd") as s_bd,      # block-diag copies done (per pair)
        nc.semaphore("s_bc") as s_bc,      # bc matmuls done (per pair)
        nc.semaphore("s_fod") as s_fod,    # fan-out done (DVE)
        nc.semaphore("s_foa") as s_foa,    # batch-1 fan-out done (ACT)
        nc.semaphore("s_out") as s_out,    # output DMAs done (8*16)
    ):
        # Input loads from the main BB so each engine dispatches them right
        # after its startup, ahead of the branch into its Block body.
        # One semaphore per DMA: a multi-DMA ring can deliver a later DMA's
        # sem-incs before an earlier DMA's data (a 1-partition transfer posts
        # dummy incs from engines with no data), so threshold counts on a
        # shared semaphore are racy.
        nc.sync.dma_start(out=wa_sb[:], in_=wa_d[:]).then_inc(s_a1, 16)
        nc.scalar.dma_start(out=wb_sb[:], in_=wb_d[:]).then_inc(s_b1, 16)
        # Mask rows + bias in one transfer, straight into rhs_bd; the mask
        # rows are multiplied by y1 in place later.
        nc.scalar.dma_start(
            out=rhs_bd[:].rearrange("p b j -> p (b j)"),
            in_=mb_d[:]).then_inc(s_mb, 16)

        block_cm = nc.Block(no_gpsimd_drain=True)
        block = block_cm.__enter__()

        W0 = KC * B  # w chunks start here in wa_sb

        def out_src(b):
            return (osb[:, b]
                    .rearrange("p c j -> p (c j)")
                    .unsqueeze(1).broadcast_to([128, NB // R, R * CS]))

        @block.tensor
        def _(pe):
            pe.wait_ge(s_a1, 16)
            for k in range(KA):
                nc.tensor.matmul(
                    y1_ps[:B, :], wa_sb[:, k * B:(k + 1) * B],
                    wa_sb[:, W0 + k * CS:W0 + (k + 1) * CS],
                    start=(k == 0), stop=False,
                )
            pe.wait_ge(s_b1, 16)
            for k in range(KA, KC):
                mm = nc.tensor.matmul(
                    y1_ps[:B, :], wa_sb[:, k * B:(k + 1) * B],
                    wb_sb[:, (k - KA) * CS:(k - KA + 1) * CS],
                    start=False, stop=(k == KC - 1),
                )
            mm.then_inc(s_pe, 1)
            pe.wait_ge(s_mb, 16)   # bias row landed
            # Pair 0 is split per batch into two PSUM banks so DVE (batch 0)
            # and ACT (batch 1) can fan out concurrently without touching the
            # same bank.
            pe.wait_ge(s_bd, 1)
            nc.tensor.matmul(
                bc_ps[:, 0, 0:1, 0:CS], ones9[:], rhs_bd[:, 0:1, :],
                start=True, stop=True,
            ).then_inc(s_bc, 1)
            nc.tensor.matmul(
                bc1_ps[:, 0:CS], ones9[:],
                rhs_bd[:, 1:2, :].rearrange("k a j -> k (a j)"),
                start=True, stop=True,
            ).then_inc(s_bc, 1)
            for p in range(1, 4):
                pe.wait_ge(s_bd, 2)  # remaining diag cols ready
                nc.tensor.matmul(
                    bc_ps[:, p, :, 0:CS], ones9[:],
                    rhs_bd[:, 2 * p:2 * p + 2, :],
                    start=True, stop=True,
                ).then_inc(s_bc, 1)

        @block.vector
        def _(dve):
            # ones9 first: its completion is ordered before PE's LDWEIGHTS
            # through the in-order DVE pipeline and the s_bd chain.
            nc.vector.memset(ones9[:], 1.0)
            dve.wait_ge(s_mb, 16)  # mask rows landed (in rhs_bd)
            dve.wait_ge(s_pe, 1)
            # Engine APs cannot start at arbitrary partitions, so the block
            # diagonal cannot be written row-by-row.  Instead build it in two
            # legal ops: rhs_bd[b, b', j] = y1[b, j] * mask[b, b', j] with
            # mask[b, b', j] = (b == b'), all partition-base-0.
            # Uneven split: pair 0's columns first so bc0 / fan0 / the first
            # output dispatch start ~0.4 us earlier; the rest in one op.
            for lo, hi in ((0, 2), (2, 8)):
                nc.vector.tensor_mul(
                    rhs_bd[0:8, lo:hi, :],
                    y1_ps[:B, :].unsqueeze(1).broadcast_to([8, hi - lo, CS]),
                    rhs_bd[0:8, lo:hi, :]).then_inc(s_bd, 1)
            dve.wait_ge(s_bc, 1)
            fan = (bc_ps[:, 0, 0:1, 0:CS]
                   .unsqueeze(2).broadcast_to([128, 1, R, CS]))
            nc.vector.tensor_copy(osb[:, 0:1], fan).then_inc(s_fod, 1)
            for p in (1, 3):   # pair 2's fan runs on ACT (its own bank)
                dve.wait_ge(s_bc, p + 2)
                fan = (bc_ps[:, p, :, 0:CS]
                       .unsqueeze(2).broadcast_to([128, 2, R, CS]))
                nc.vector.tensor_copy(osb[:, 2 * p:2 * p + 2], fan).then_inc(
                    s_fod, 1)

        # Output batches alternate between the two HWDGE rings (SP: even,
        # ACT: odd): a single ring only sustains ~184 GB/s (each SDMA engine
        # serves each queue at ~11.5 B/ns), so the full ~368 GB/s needs both
        # rings busy for the whole drain.
        @block.scalar
        def _(act):
            act.wait_ge(s_bc, 2)
            fan = (bc1_ps[:, 0:CS].unsqueeze(1).unsqueeze(2)
                   .broadcast_to([128, 1, R, CS]))
            nc.scalar.copy(osb[:, 1:2], fan).then_inc(s_foa, 1)
            # wait on the copy's semaphore: program order does not order a
            # dma_start's reads after a prior compute op's writes.
            act.wait_ge(s_foa, 1)
            act.dma_start(out=out_d[1], in_=out_src(1)).then_inc(s_out, 16)
            act.wait_ge(s_bc, 4)
            fan = (bc_ps[:, 2, :, 0:CS]
                   .unsqueeze(2).broadcast_to([128, 2, R, CS]))
            nc.scalar.copy(osb[:, 4:6], fan).then_inc(s_foa, 1)
            act.wait_ge(s_fod, 2)   # DVE fan1 done -> batches 2,3
            act.dma_start(out=out_d[3], in_=out_src(3)).then_inc(s_out, 16)
            act.wait_ge(s_foa, 2)   # own fan2 done -> batches 4,5
            act.dma_start(out=out_d[5], in_=out_src(5)).then_inc(s_out, 16)
            act.wait_ge(s_fod, 3)   # DVE fan3 done -> batches 6,7
            # b6 here / b7 on SP: ACT's fans put it ~0.9 us behind SP by the
            # 4th dispatch, so swapping the last pair lands both rings' final
            # DMAs together instead of running the drain tail single-queue.
            act.dma_start(out=out_d[6], in_=out_src(6)).then_inc(s_out, 16)

        @block.sync
        def _(sp):
            sp.wait_ge(s_fod, 1)
            sp.dma_start(out=out_d[0], in_=out_src(0)).then_inc(s_out, 16)
            sp.wait_ge(s_fod, 2)
            sp.dma_start(out=out_d[2], in_=out_src(2)).then_inc(s_out, 16)
            sp.wait_ge(s_foa, 2)
            sp.dma_start(out=out_d[4], in_=out_src(4)).then_inc(s_out, 16)
            sp.wait_ge(s_fod, 3)
            sp.dma_start(out=out_d[7], in_=out_src(7)).then_inc(s_out, 16)
            sp.wait_ge(s_out, 128)

        # Block exit emits per-engine drains + an all-engine barrier.  The
        # barrier's butterfly costs ~3.5 us of semaphore ping-pong and is
        # redundant here: every semaphore's final value happens-before
        # s_out>=128 through the dependency chain, so SP's wait alone orders
        # the trailing sem_clear.  Suppress the barrier, keep the drains.
        with mock.patch.object(bass.Bass, "all_engine_barrier",
                               lambda self, **kw: None):
            block_cm.__exit__(None, None, None)
        sems = [s_a1, s_b1, s_mb, s_pe, s_bd, s_bc, s_fod, s_foa, s_out]
        nums = sorted(s.num for s in sems)
        assert nums == list(range(nums[0], nums[0] + len(nums)))
        nc.sync.sem_clear(range(nums[0], nums[-1] + 1))

    nc.compile()
    return nc


def _prep_inputs(spe_agg, W_proj, b_proj):
    # spe_host[p, k*B+b] = spe_agg[b, k*128+p]
    spe_host = (np.ascontiguousarray(spe_agg.T).reshape(KC, 128, B)
                .transpose(1, 0, 2).astype(IN_NP).reshape(128, KC * B))

    # mask[b, b'*CS + j] = (b == b')
    mask = np.kron(np.eye(B, dtype=np.float32),
                   np.ones((1, CS), dtype=np.float32)).astype(IN_NP)
    mask = np.ascontiguousarray(mask)

    wpt_full = np.ascontiguousarray(W_proj.T)          # (C, C): [c, j]
    in_maps = []
    for i in range(N_CORES):
        j0 = i * CS
        w = (wpt_full[:, j0:j0 + CS].reshape(KC, 128, CS)
             .transpose(1, 0, 2))                       # (128, KC, CS)
        wa = np.concatenate(
            [spe_host, w[:, :KA].reshape(128, KA * CS).astype(IN_NP)], axis=1)
        wb = np.ascontiguousarray(
            w[:, KA:].reshape(128, W2_COLS)).astype(IN_NP)
        mb = np.concatenate(
            [mask, np.tile(b_proj[j0:j0 + CS], B)[None, :].astype(IN_NP)],
            axis=0)
        in_maps.append({"wa": np.ascontiguousarray(wa),
                        "wb": wb, "mb": np.ascontiguousarray(mb)})
    return in_maps


def kernel(x, spe_agg, W_qkv, W_proj, b_proj):
    # x and W_qkv do not affect the output (see module analysis above).
    spe_agg = np.ascontiguousarray(spe_agg, dtype=np.float32)
    W_proj = np.ascontiguousarray(W_proj, dtype=np.float32)
    b_proj = np.ascontiguousarray(b_proj, dtype=np.float32)

    if "nc" not in _CACHE:
        _CACHE["nc"] = _build()
    nc = _CACHE["nc"]

    in_maps = _prep_inputs(spe_agg, W_proj, b_proj)
    res = run_bass_kernel_spmd(nc, in_maps, core_ids=list(range(N_CORES)))
    # per-core out: (B, 128, NB, CS) with row n = p*8 + r -> (B, N, CS).
    # Device writes bf16; the values are exactly bf16-representable, so the
    # host f32 upcast is lossless.
    shards = [np.asarray(res.results[i]["out"]).astype(np.float32)
              .reshape(B, N, CS) for i in range(N_CORES)]
    return np.concatenate(shards, axis=2)
